# revision 1
# baseline (speedup 1.0000x reference)
"""Multi-head dot-product attention (B=2, S=2048, D=2048, H=16, HD=128) with
RoPE + causal mask, sharded over 8 NeuronCores: batch (2) x head-groups (4).

Each core computes 4 heads of one batch element end-to-end (QKV projections,
RoPE, causal softmax attention, output projection); the host sums the two
head-group partials per batch element.

Self-contained: hardcodes all shapes; builds/compiles the Bass program once
per process and runs it via run_bass_kernel_spmd on cores 0-7.
"""

import os
import sys
import types

import ml_dtypes
import numpy as np

B, S, D, H, HD = 2, 2048, 2048, 16, 128
HPC = 4                 # heads per core
HW = HPC * HD           # 512: per-core projection width
NQB = S // 512          # 4 query blocks / token quarters of 512
NKT = S // 128          # 16 key-token tiles of 128
NDC = D // 128          # 16 contraction chunks of 128
N_CORES = 8
SCALE = float(HD) ** -0.5

BF16 = ml_dtypes.bfloat16

_CACHE = {}


def _install_ntff_hook():
    """The image's antenv lacks axon_hooks, so boot() couldn't register the
    NTFF profile hook; recreate the module + hook so trace=True works."""
    if "antenv.axon_hooks" in sys.modules:
        return
    try:
        import antenv  # noqa: F401
        mod = types.ModuleType("antenv.axon_hooks")
        _h = [None]
        mod.set_axon_ntff_profile_hook = lambda h: _h.__setitem__(0, h)
        mod.get_axon_ntff_profile_hook = lambda: _h[0]
        sys.modules["antenv.axon_hooks"] = mod
        from trn_agent_boot.trn_boot import _ntff_profile_via_ctypes
        mod.set_axon_ntff_profile_hook(
            _ntff_profile_via_ctypes("/opt/axon/libaxon_pjrt.so"))
    except Exception:
        pass


def _build():
    import concourse.mybir as mybir
    import concourse.tile as tile
    from concourse import bacc
    from concourse import bass_isa

    f32 = mybir.dt.float32
    bf16 = mybir.dt.bfloat16
    fp16 = mybir.dt.float16
    Exp = mybir.ActivationFunctionType.Exp

    nc = bacc.Bacc("TRN2", target_bir_lowering=False, debug=False,
                   enable_asserts=True, num_devices=N_CORES)

    dram = {}
    for name, shape, dt in [
        ("xqT", [D, S], bf16), ("xkvT", [D, S], bf16),
        ("wq", [D, HW], bf16), ("wk", [D, HW], bf16), ("wv", [D, HW], bf16),
        ("wo", [HW, D], bf16),
        ("sinT", [HD, S], bf16), ("cosT", [HD, S], bf16),
        ("rmatT", [HD, HD], bf16),
        ("ones_col", [128, 1], fp16), ("ones_row", [1, 128], fp16),
        ("maskt", [128, 4 * 512], fp16),
    ]:
        dram[name] = nc.dram_tensor(name, shape, dt, kind="ExternalInput").ap()
    outp = nc.dram_tensor("outp", [S, D], f32, kind="ExternalOutput").ap()

    with tile.TileContext(nc) as tc:
        with (
            tc.tile_pool(name="const", bufs=1) as cpool,
            tc.tile_pool(name="kt", bufs=1) as kt_pool,
            tc.tile_pool(name="qt", bufs=1) as qt_pool,
            tc.tile_pool(name="vsb", bufs=1) as v_pool,
            tc.tile_pool(name="ctxn", bufs=1) as ctx_pool,
            tc.tile_pool(name="wkv", bufs=1) as wkv_pool,
            tc.tile_pool(name="xin", bufs=2) as xpool,
            tc.tile_pool(name="raw", bufs=3) as raw_pool,
            tc.tile_pool(name="t12", bufs=4) as t12_pool,
            tc.tile_pool(name="pp", bufs=8) as ppool,
            tc.tile_pool(name="sacc", bufs=6) as sacc_pool,
            tc.tile_pool(name="rs", bufs=2) as rs_pool,
            tc.tile_pool(name="rcp", bufs=3) as rpool,
            tc.tile_pool(name="osb", bufs=6) as opool,
            # one PSUM pool for the whole kernel: 4 tags x 2 bufs = 8 banks;
            # no pool-boundary stalls between phases
            tc.tile_pool(name="ps", space="PSUM", bufs=2) as pspool,
        ):
            def load_chunks(pool, name, nch, width, tag=None, eng=None):
                # [nch*128, width] dram -> one [128, nch*width] sbuf tile,
                # two chunks per DMA (strided AP) to halve issue overhead
                t = pool.tile([128, nch * width], bf16, tag=tag or name,
                              name=name + "_sb")
                dv = dram[name].rearrange("(n p) w -> p n w", p=128)
                step = 4 if nch % 4 == 0 else 2
                for i in range(0, nch, step):
                    e = eng or nc.sync
                    e.dma_start(t[:, i * width:(i + step) * width],
                                dv[:, i:i + step, :])
                return t

            def load(name, shape, dt=bf16):
                t = cpool.tile(shape, dt, tag=name, name=name)
                nc.scalar.dma_start(t[:], dram[name][:])
                return t

            # startup: three DMA queues in parallel so the first projection
            # matmuls start ASAP: wk on Scalar, xkvT q0 on Sync, wv on GpSimd
            wk_sb = load_chunks(wkv_pool, "wk", NDC, HW, eng=nc.scalar)
            rmatT = load("rmatT", [HD, HD])
            wv_sb = load_chunks(wkv_pool, "wv", NDC, HW, eng=nc.scalar)
            sinT = load("sinT", [HD, S])
            cosT = load("cosT", [HD, S])
            maskt = load("maskt", [128, 4 * 512], fp16)
            ones_col = load("ones_col", [128, 1], fp16)
            ones_row = load("ones_row", [1, 128], fp16)
            wo_sb = load_chunks(cpool, "wo", HW // 128, D, eng=nc.scalar)

            # per-head projection outputs (+rope for Q/K)
            kt_sb = [kt_pool.tile([128, S], bf16, tag=f"kt{h}", name=f"kt{h}")
                     for h in range(HPC)]
            qt_sb = [qt_pool.tile([128, S], bf16, tag=f"qt{h}", name=f"qt{h}")
                     for h in range(HPC)]
            v_sb = v_pool.tile([128, NKT * HW], fp16, tag="v", name="v_sb")
            ctx_sb = [ctx_pool.tile([128, S], bf16, tag=f"ctx{h}",
                                    name=f"ctx{h}") for h in range(HPC)]

            def proj_quarter(xname, tq, w_sb, out_tiles, acc_tag, rot_tag,
                             with_v=False):
                """Token-quarter tq of x^T: per-head 512-wide projection
                (+rope) into out_tiles[h][:, tq*512:...]; for the kv pass
                also the 4 V token-tiles of this quarter."""
                sl = slice(tq * 512, (tq + 1) * 512)
                xt = xpool.tile([128, NDC * 512], bf16, tag="xin",
                                name=f"{xname}_{tq}")
                xv = dram[xname].rearrange("(n p) s -> p n s", p=128)
                for kc in range(0, NDC, 4):
                    # one DMA per 4-chunk group: [128, 4, 512] strided AP
                    nc.sync.dma_start(
                        xt[:, kc * 512:(kc + 4) * 512],
                        xv[:, kc:kc + 4, tq * 512:(tq + 1) * 512])
                for h in range(HPC):
                    ps = pspool.tile([128, 512], f32, tag=acc_tag,
                                     name=f"ps_{xname}_{tq}_{h}")
                    for kc in range(NDC):
                        nc.tensor.matmul(
                            ps[:],
                            lhsT=w_sb[:, kc * HW + h * HD:
                                      kc * HW + (h + 1) * HD],
                            rhs=xt[:, kc * 512:(kc + 1) * 512],
                            start=(kc == 0), stop=(kc == NDC - 1))
                    raw = raw_pool.tile([128, 512], bf16, tag="raw",
                                        name=f"raw_{xname}_{tq}_{h}")
                    nc.scalar.copy(raw[:], ps[:])
                    # rope: out = raw*cos + (R @ raw)*sin
                    rot = pspool.tile([128, 512], f32, tag=rot_tag,
                                      name=f"rot_{xname}_{tq}_{h}")
                    nc.tensor.matmul(rot[:], lhsT=rmatT[:], rhs=raw[:])
                    t1 = t12_pool.tile([128, 512], bf16, tag="t1", name="t1")
                    nc.vector.tensor_mul(t1[:], rot[:], sinT[:, sl])
                    t2 = t12_pool.tile([128, 512], bf16, tag="t2", name="t2")
                    nc.vector.tensor_mul(t2[:], raw[:], cosT[:, sl])
                    nc.vector.tensor_add(out_tiles[h][:, sl], t1[:], t2[:])
                if with_v:
                    emit_v(tq, xt)
                return xt

            def emit_v(tq, xt):
                for ti in range(4):
                    t = tq * 4 + ti
                    ps = pspool.tile([128, 512], f32, tag="C",
                                     name=f"vps_{t}")
                    for kc in range(NDC):
                        nc.tensor.matmul(
                            ps[:],
                            lhsT=xt[:, kc * 512 + ti * 128:
                                    kc * 512 + (ti + 1) * 128],
                            rhs=wv_sb[:, kc * HW:(kc + 1) * HW],
                            start=(kc == 0), stop=(kc == NDC - 1))
                    nc.scalar.copy(v_sb[:, t * HW:(t + 1) * HW], ps[:])

            def attention_block(qb):
                """Causal attention for query block qb, heads in pairs so
                independent work hides softmax chains; PSUM: ctx=A, st=C,
                s/rb/WO=D (B is left for the concurrent Q-projection)."""
                qsl = slice(qb * 512, (qb + 1) * 512)
                last = 4 * qb + 3
                for hp in range(2):
                    pair = (2 * hp, 2 * hp + 1)
                    ctx_ps = {h: pspool.tile([128, 512], f32, tag="A",
                                             name=f"ctxps_{h}_{qb}")
                              for h in pair}
                    accs = {h: sacc_pool.tile([128, 512], fp16, tag="acc",
                                              name=f"acc_{h}_{qb}")
                            for h in pair}
                    for kt in range(last + 1):
                        for h in pair:
                            # columns left of the causal diagonal are never
                            # valid: compute S/exp/PV only on [off:512] and
                            # mask just the 128-wide triangular sub-block
                            off = 128 * (kt - 4 * qb) if kt >= 4 * qb else 0
                            st = pspool.tile([128, 512], f32, tag="C",
                                             name=f"st_{h}_{qb}_{kt}")
                            nc.tensor.matmul(
                                st[:, off:],
                                lhsT=kt_sb[h][:, kt * 128:(kt + 1) * 128],
                                rhs=qt_sb[h][:, qb * 512 + off:
                                             (qb + 1) * 512])
                            p = ppool.tile([128, 512], fp16, tag="p",
                                           name=f"p_{h}_{qb}_{kt}")
                            nc.scalar.activation(p[:, off:], st[:, off:],
                                                 Exp, scale=SCALE)
                            if kt >= 4 * qb:
                                nc.vector.tensor_mul(
                                    p[:, off:off + 128],
                                    p[:, off:off + 128], maskt[:, 0:128])
                            if kt == 0:
                                nc.vector.tensor_copy(accs[h][:], p[:])
                            else:
                                nc.vector.tensor_add(accs[h][:, off:],
                                                     accs[h][:, off:],
                                                     p[:, off:])
                            nc.tensor.matmul(
                                ctx_ps[h][:, off:],
                                lhsT=v_sb[:, kt * HW + h * HD:
                                          kt * HW + (h + 1) * HD],
                                rhs=p[:, off:], start=(kt == 0),
                                stop=(kt == last))
                    for h in pair:
                        s_ps = pspool.tile([1, 512], f32, tag="D",
                                           name=f"sps_{h}_{qb}")
                        nc.tensor.matmul(s_ps[:], lhsT=ones_col[:],
                                         rhs=accs[h][:])
                        s_sb = rs_pool.tile([1, 512], fp16, tag="ssb",
                                            name=f"ssb_{h}_{qb}")
                        nc.scalar.copy(s_sb[:], s_ps[:])
                        rb_ps = pspool.tile([128, 512], f32, tag="D",
                                            name=f"rbps_{h}_{qb}")
                        nc.tensor.matmul(rb_ps[:], lhsT=ones_row[:],
                                         rhs=s_sb[:])
                        rb_sb = rpool.tile([128, 512], f32, tag="rb",
                                           name=f"rbsb_{h}_{qb}")
                        nc.vector.reciprocal_approx_fast(rb_sb[:], rb_ps[:])
                        nc.vector.tensor_mul(ctx_sb[h][:, qsl],
                                             ctx_ps[h][:], rb_sb[:])

            def wo_block(qb):
                for qt in range(qb * 4, qb * 4 + 4):
                    for db in range(NQB):
                        ps = pspool.tile([128, 512], f32, tag="D",
                                         name=f"ops_{qt}_{db}")
                        for h in range(HPC):
                            nc.tensor.matmul(
                                ps[:],
                                lhsT=ctx_sb[h][:, qt * 128:(qt + 1) * 128],
                                rhs=wo_sb[:, h * D + db * 512:
                                          h * D + (db + 1) * 512],
                                start=(h == 0), stop=(h == HPC - 1))
                        osb = opool.tile([128, 512], f32, tag="o",
                                         name=f"osb_{qt}_{db}")
                        nc.vector.tensor_copy(osb[:], ps[:])
                        nc.sync.dma_start(
                            outp[qt * 128:(qt + 1) * 128,
                                 db * 512:(db + 1) * 512], osb[:])

            # ---- phase 1: K^T + V (stream xkvT) ----
            for tq in range(NQB):
                proj_quarter("xkvT", tq, wk_sb, kt_sb, "A", "B", with_v=True)
            # ---- phase 2: Q^T quarters interleaved with attention + WO:
            # the projection's dense matmuls (tag B) fill PE gaps left by the
            # exp-gated attention stream; WO lags one block so WO(q2) fills
            # attention(q3).
            wq_sb = load_chunks(wkv_pool, "wq", NDC, HW, tag="wk")
            for tq in reversed(range(NQB)):
                proj_quarter("xqT", tq, wq_sb, qt_sb, "B", "B")
                attention_block(tq)
                wo_block(tq)

    nc.compile()
    return nc


def _host_constants():
    # sin/cos tables exactly as the flaxformer reference (fp32 math)
    fraction = np.arange(0, HD, 2, dtype=np.float32) / np.float32(HD)
    timescale = (np.float32(10000.0) ** fraction).astype(np.float32)
    sinusoid = np.einsum(
        "i,j->ij", np.arange(S, dtype=np.float32),
        (np.float32(1.0) / timescale)).astype(np.float32)
    sinusoid = np.concatenate([sinusoid, sinusoid], axis=-1)  # [S, HD]
    sinT = np.sin(sinusoid).astype(np.float32).T.copy()
    cosT = np.cos(sinusoid).astype(np.float32).T.copy()

    # rotate_half as a matmul: rot = R @ x, lhsT = R^T
    R = np.zeros((HD, HD), np.float32)
    for i in range(64):
        R[i, i + 64] = -1.0
        R[i + 64, i] = 1.0

    # causal mask variants for the 4 diagonal sub-blocks: allowed iff
    # q - k >= 0 with q = 512*qb + c, k = 128*(4*qb + v) + r
    r = np.arange(128)[:, None]
    c = np.arange(512)[None, :]
    maskt = np.concatenate(
        [(c - r >= 128 * v).astype(np.float32) for v in range(4)], axis=1)

    return {
        "sinT": sinT.astype(BF16), "cosT": cosT.astype(BF16),
        "rmatT": R.T.copy().astype(BF16),
        "ones_col": np.ones((128, 1), np.float16),
        "ones_row": np.ones((1, 128), np.float16),
        "maskt": maskt.astype(np.float16),
    }


def kernel(inputs_q, inputs_kv, wq, wk, wv, wo, mask=None):
    _install_ntff_hook()
    from concourse import bass_utils

    if "nc" not in _CACHE:
        _CACHE["nc"] = _build()
        _CACHE["consts"] = _host_constants()
    nc = _CACHE["nc"]
    consts = _CACHE["consts"]

    wq2 = np.asarray(wq, np.float32).reshape(D, H * HD)
    wk2 = np.asarray(wk, np.float32).reshape(D, H * HD)
    wv2 = np.asarray(wv, np.float32).reshape(D, H * HD)
    wo2 = np.asarray(wo, np.float32).reshape(H * HD, D)
    xq = np.asarray(inputs_q, np.float32)
    xkv = np.asarray(inputs_kv, np.float32)

    in_maps = []
    for c in range(N_CORES):
        b, hg = divmod(c, H // HPC)
        hs = slice(hg * HW, (hg + 1) * HW)
        in_maps.append({
            "xqT": np.ascontiguousarray(xq[b].T).astype(BF16),
            "xkvT": np.ascontiguousarray(xkv[b].T).astype(BF16),
            "wq": wq2[:, hs].astype(BF16),
            "wk": wk2[:, hs].astype(BF16),
            "wv": wv2[:, hs].astype(BF16),
            "wo": wo2[hs, :].astype(BF16),
            **consts,
        })

    trace = bool(int(os.environ.get("KERNEL_TRACE", "0")))
    res = bass_utils.run_bass_kernel_spmd(
        nc, in_maps, core_ids=list(range(N_CORES)), trace=trace)
    _CACHE["last_result"] = res

    out = np.zeros((B, S, D), np.float32)
    for c in range(N_CORES):
        out[c // (H // HPC)] += res.results[c]["outp"]
    return out



# revision 9
# speedup vs baseline: 1.1330x; 1.1330x over previous
"""Multi-head dot-product attention (B=2, S=2048, D=2048, H=16, HD=128) with
RoPE + causal mask, sharded over 8 NeuronCores: batch (2) x head-groups (4).

Each core computes 4 heads of one batch element end-to-end; the host sums the
four head-group partials per batch element.

Mixed precision: all K/V (and queries >= 512) run fp8 e4m3 with DoubleRow
matmuls (2 contraction chunks per instruction) for the QKV projections, PV,
softmax sums, and the output projection; query rows 0-511 additionally run a
bf16 path end-to-end (bf16 K/V/logits/WO for those rows) so the large-
magnitude early-token outputs stay accurate. exp on the fp8 path carries a -2
bias so probabilities stay inside e4m3 range (normalization cancels it).
rotate_half is done by two SBUF->SBUF partition-swap DMAs with the sign
folded into the sin table. Softmax sums accumulate on the tensor engine via
an all-ones lhsT matmul into a broadcast PSUM tile. Validated vs the fp32
reference: rel err ~1e-2 (budget 2e-2).

Self-contained: hardcodes all shapes; builds/compiles the Bass program once
per process and runs it via run_bass_kernel_spmd on cores 0-7.
"""

import os
import sys
import types

import ml_dtypes
import numpy as np

B, S, D, H, HD = 2, 2048, 2048, 16, 128
HPC = 4                 # heads per core
HW = HPC * HD           # 512: per-core projection width
NDC = D // 128          # 16 contraction chunks of 128
NPR = NDC // 2          # 8 fp8 DoubleRow chunk-pairs
N_CORES = 8
SCALE = float(HD) ** -0.5
EXPB = -2.0             # exp bias on the fp8 path (e4m3 max normal is 240)

BF16 = ml_dtypes.bfloat16
F8 = ml_dtypes.float8_e4m3

_CACHE = {}


def _install_ntff_hook():
    """The image's antenv lacks axon_hooks, so boot() couldn't register the
    NTFF profile hook; recreate the module + hook so trace=True works."""
    if "antenv.axon_hooks" in sys.modules:
        return
    try:
        import antenv  # noqa: F401
        mod = types.ModuleType("antenv.axon_hooks")
        _h = [None]
        mod.set_axon_ntff_profile_hook = lambda h: _h.__setitem__(0, h)
        mod.get_axon_ntff_profile_hook = lambda: _h[0]
        sys.modules["antenv.axon_hooks"] = mod
        from trn_agent_boot.trn_boot import _ntff_profile_via_ctypes
        mod.set_axon_ntff_profile_hook(
            _ntff_profile_via_ctypes("/opt/axon/libaxon_pjrt.so"))
    except Exception:
        pass


def _weave(*streams):
    """Round-robin the generators until all are exhausted: interleaves their
    instruction issue so per-engine queues stay balanced in program order."""
    live = list(streams)
    while live:
        nxt = []
        for g in live:
            try:
                next(g)
                nxt.append(g)
            except StopIteration:
                pass
        live = nxt


def _drain(g):
    for _ in g:
        pass


def _build():
    import concourse.mybir as mybir
    import concourse.tile as tile
    from concourse import bacc

    f32 = mybir.dt.float32
    bf16 = mybir.dt.bfloat16
    fp16 = mybir.dt.float16
    fp8 = mybir.dt.float8e4
    Exp = mybir.ActivationFunctionType.Exp
    DR = mybir.MatmulPerfMode.DoubleRow

    nc = bacc.Bacc("TRN2", target_bir_lowering=False, debug=False,
                   enable_asserts=True, num_devices=N_CORES)

    dram = {}
    for name, shape, dt in [
        ("xq_bf", [D, 512], bf16), ("xq_f8", [D, 1536], fp8),
        ("xkv_bf", [D, 512], bf16), ("xkv_f8", [D, S], fp8),
        ("wq_bf", [D, HW], bf16), ("wq_f8", [D, HW], fp8),
        ("wk_bf", [D, HW], bf16), ("wk_f8", [D, HW], fp8),
        ("wv_bf", [D, HW], bf16), ("wv_f8", [D, HW], fp8),
        ("wo_bf", [HW, D], bf16), ("wo_f8", [HW, D], fp8),
        ("sinN", [HD, S], bf16),    # sin with rows 0-63 negated (rot fold)
        ("cosT", [HD, S], bf16),
        ("ones16", [128, 128], fp16),
        ("ones8", [128, 2, 128], fp8),
        ("tri", [128, 128], fp16), ("ztri", [128, 256], fp16),
    ]:
        dram[name] = nc.dram_tensor(name, shape, dt, kind="ExternalInput").ap()
    outp = nc.dram_tensor("outp", [S, D], bf16, kind="ExternalOutput").ap()

    with tile.TileContext(nc) as tc:
        with (
            tc.tile_pool(name="const", bufs=1) as cpool,
            tc.tile_pool(name="wts", bufs=1) as wpool,
            tc.tile_pool(name="kq", bufs=1) as kq_pool,
            tc.tile_pool(name="vsb", bufs=1) as v_pool,
            tc.tile_pool(name="ctxn", bufs=1) as ctx_pool,
            tc.tile_pool(name="xbf", bufs=1) as xbf_pool,
            tc.tile_pool(name="xf8", bufs=2) as xf8_pool,
            tc.tile_pool(name="raw", bufs=3) as raw_pool,
            tc.tile_pool(name="rot", bufs=3) as rot_pool,
            tc.tile_pool(name="t12", bufs=2) as t12_pool,
            tc.tile_pool(name="pbf", bufs=3) as pbf_pool,
            tc.tile_pool(name="p8", bufs=4) as p8_pool,
            tc.tile_pool(name="rcp", bufs=2) as rpool,
            tc.tile_pool(name="osb", bufs=4) as opool,
            tc.tile_pool(name="ps", space="PSUM", bufs=1) as pspool,
        ):
            def loadw(name, nch, width, dt, eng, ndma):
                """[nch*128, width] dram -> [128, nch, width] sbuf tile."""
                t = wpool.tile([128, nch, width], dt, tag=name, name=name)
                dv = dram[name].rearrange("(n p) w -> p n w", p=128)
                step = nch // ndma
                for i in range(0, nch, step):
                    eng.dma_start(t[:, i:i + step, :], dv[:, i:i + step, :])
                return t

            def loadc(name, shape, dt, eng):
                t = cpool.tile(shape, dt, tag=name, name=name)
                eng.dma_start(t[:], dram[name][:])
                return t

            # ---- startup DMAs: fp8 K/V weights first so compute starts
            # immediately; bf16 weights stream in behind them ----
            wk_f8 = loadw("wk_f8", NDC, HW, fp8, nc.gpsimd, 2)
            wv_f8 = loadw("wv_f8", NDC, HW, fp8, nc.gpsimd, 2)
            sinN = loadc("sinN", [HD, S], bf16, nc.scalar)
            cosT = loadc("cosT", [HD, S], bf16, nc.scalar)
            wq_f8 = loadw("wq_f8", NDC, HW, fp8, nc.gpsimd, 2)
            ones16 = loadc("ones16", [128, 128], fp16, nc.scalar)
            ones8 = loadc("ones8", [128, 2, 128], fp8, nc.scalar)
            tri = loadc("tri", [128, 128], fp16, nc.scalar)
            ztri = loadc("ztri", [128, 256], fp16, nc.scalar)
            ebias = cpool.tile([128, 1], f32, tag="ebias", name="ebias")
            nc.gpsimd.memset(ebias[:], EXPB)
            wk_bf = loadw("wk_bf", NDC, HW, bf16, nc.scalar, 1)
            wv_bf = loadw("wv_bf", NDC, HW, bf16, nc.scalar, 1)
            wq_bf = loadw("wq_bf", NDC, HW, bf16, nc.scalar, 1)
            wo_f8_sb = cpool.tile([128, HPC, D], fp8, tag="wo_f8",
                                  name="wo_f8")
            dv = dram["wo_f8"].rearrange("(n p) d -> p n d", p=128)
            nc.gpsimd.dma_start(wo_f8_sb[:], dv[:])
            wo_bf_sb = cpool.tile([128, HPC, D], bf16, tag="wo_bf",
                                  name="wo_bf")
            dv = dram["wo_bf"].rearrange("(n p) d -> p n d", p=128)
            nc.scalar.dma_start(wo_bf_sb[:], dv[:])

            # persistent per-head projection outputs
            kt_bf = [kq_pool.tile([128, 512], bf16, tag=f"ktb{h}",
                                  name=f"ktb{h}") for h in range(HPC)]
            kt_f8 = [kq_pool.tile([128, S], fp8, tag=f"kt8{h}",
                                  name=f"kt8{h}") for h in range(HPC)]
            qt_bf = [kq_pool.tile([128, 512], bf16, tag=f"qtb{h}",
                                  name=f"qtb{h}") for h in range(HPC)]
            qt_f8 = [kq_pool.tile([128, 1536], fp8, tag=f"qt8{h}",
                                  name=f"qt8{h}") for h in range(HPC)]
            v_bf = v_pool.tile([128, 4, 512], fp16, tag="vbf", name="v_bf")
            v_f8 = v_pool.tile([128, 16, 512], fp8, tag="vf8", name="v_f8")
            ctx_bf = [ctx_pool.tile([128, 512], bf16, tag=f"cxb{h}",
                                    name=f"cxb{h}") for h in range(HPC)]
            ctx_f8 = ctx_pool.tile([128, HPC, 1536], fp8, tag="cx8",
                                   name="ctx_f8")

            rot_dma = [nc.gpsimd, nc.gpsimd]

            def rope(ps_acc, tq, out_ap, ridx):
                """out = raw*cos + rot(raw)*sinN for token quarter tq.
                rotate_half = partition swap via 2 SBUF->SBUF DMAs (the sign
                of the top half is folded into sinN)."""
                sl = slice(tq * 512, (tq + 1) * 512)
                raw = raw_pool.tile([128, 512], bf16, tag="raw", name="raw")
                nc.scalar.copy(raw[:], ps_acc[:])
                rot = rot_pool.tile([128, 512], bf16, tag="rot", name="rot")
                eng = rot_dma[ridx % 2]
                eng.dma_start(rot[0:64, :], raw[64:128, :])
                eng.dma_start(rot[64:128, :], raw[0:64, :])
                t1 = t12_pool.tile([128, 512], bf16, tag="t1", name="t1")
                nc.vector.tensor_mul(t1[:], rot[:], sinN[:, sl])
                t2 = t12_pool.tile([128, 512], bf16, tag="t2", name="t2")
                nc.vector.tensor_mul(t2[:], raw[:], cosT[:, sl])
                nc.vector.tensor_add(out_ap, t1[:], t2[:])

            def load_x_bf(dname):
                xt = xbf_pool.tile([128, NDC, 512], bf16, tag="xbf",
                                   name=dname)
                xv = dram[dname].rearrange("(n p) s -> p n s", p=128)
                for c in range(0, NDC, 4):
                    nc.sync.dma_start(xt[:, c:c + 4, :], xv[:, c:c + 4, :])
                return xt

            def load_x_f8(dname, col0):
                xt = xf8_pool.tile([128, NDC, 512], fp8, tag="xf8",
                                   name=f"{dname}_{col0}")
                xv = dram[dname].rearrange("(n p) s -> p n s", p=128)
                sl = slice(col0, col0 + 512)
                for c in range(0, NDC, 8):
                    nc.sync.dma_start(xt[:, c:c + 8, :], xv[:, c:c + 8, sl])
                return xt

            def kv_f8(tq):
                """K+V projection of token quarter tq (0..3), fp8 DoubleRow."""
                xt = load_x_f8("xkv_f8", tq * 512)
                for h in range(HPC):
                    ps = pspool.tile([128, 512], f32, tag=f"B{h % 2}",
                                     name=f"k{tq}_{h}")
                    for c in range(NPR):
                        nc.tensor.matmul(
                            ps[:],
                            lhsT=wk_f8[:, 2 * c:2 * c + 2,
                                       h * HD:(h + 1) * HD],
                            rhs=xt[:, 2 * c:2 * c + 2, :], start=(c == 0),
                            stop=(c == NPR - 1), perf_mode=DR)
                    rope(ps, tq, kt_f8[h][:, tq * 512:(tq + 1) * 512], h)
                    yield
                for ti in range(4):
                    ps = pspool.tile([128, 512], f32, tag=f"D{ti % 2}",
                                     name=f"v{tq}_{ti}")
                    for c in range(NPR):
                        nc.tensor.matmul(
                            ps[:],
                            lhsT=xt[:, 2 * c:2 * c + 2,
                                    ti * 128:(ti + 1) * 128],
                            rhs=wv_f8[:, 2 * c:2 * c + 2, :], start=(c == 0),
                            stop=(c == NPR - 1), perf_mode=DR)
                    nc.scalar.copy(v_f8[:, tq * 4 + ti, :], ps[:])
                    yield

            def kv0_bf():
                """bf16 K+V projection of quarter 0 (for query rows 0-511)."""
                xt = load_x_bf("xkv_bf")
                for h in range(HPC):
                    ps = pspool.tile([128, 512], f32, tag=f"B{h % 2}",
                                     name=f"k0b_{h}")
                    for c in range(NDC):
                        nc.tensor.matmul(
                            ps[:], lhsT=wk_bf[:, c, h * HD:(h + 1) * HD],
                            rhs=xt[:, c, :], start=(c == 0),
                            stop=(c == NDC - 1))
                    rope(ps, 0, kt_bf[h][:], h)
                    yield
                for ti in range(4):
                    ps = pspool.tile([128, 512], f32, tag=f"D{ti % 2}",
                                     name=f"v0b_{ti}")
                    for c in range(NDC):
                        nc.tensor.matmul(
                            ps[:], lhsT=xt[:, c, ti * 128:(ti + 1) * 128],
                            rhs=wv_bf[:, c, :], start=(c == 0),
                            stop=(c == NDC - 1))
                    nc.scalar.copy(v_bf[:, ti, :], ps[:])
                    yield

            def q_f8(tq):
                """fp8 Q projection of quarter tq in 1..3."""
                xt = load_x_f8("xq_f8", (tq - 1) * 512)
                for h in range(HPC):
                    ps = pspool.tile([128, 512], f32, tag=f"B{h % 2}",
                                     name=f"q{tq}_{h}")
                    for c in range(NPR):
                        nc.tensor.matmul(
                            ps[:],
                            lhsT=wq_f8[:, 2 * c:2 * c + 2,
                                       h * HD:(h + 1) * HD],
                            rhs=xt[:, 2 * c:2 * c + 2, :], start=(c == 0),
                            stop=(c == NPR - 1), perf_mode=DR)
                    rope(ps, tq,
                         qt_f8[h][:, (tq - 1) * 512:tq * 512], h)
                    yield

            def q0_bf():
                xt = load_x_bf("xq_bf")
                for h in range(HPC):
                    ps = pspool.tile([128, 512], f32, tag=f"B{h % 2}",
                                     name=f"q0b_{h}")
                    for c in range(NDC):
                        nc.tensor.matmul(
                            ps[:], lhsT=wq_bf[:, c, h * HD:(h + 1) * HD],
                            rhs=xt[:, c, :], start=(c == 0),
                            stop=(c == NDC - 1))
                    rope(ps, 0, qt_bf[h][:], h)
                    yield

            def attn_bf():
                """Causal attention for query rows 0-511, bf16/fp16; softmax
                sum accumulates on the PE via an all-ones lhsT matmul."""
                for h in range(HPC):
                    ctx_ps = pspool.tile([128, 512], f32, tag="A",
                                         name=f"actx_{h}")
                    rb_ps = pspool.tile([128, 512], f32, tag="R",
                                        name=f"arb_{h}")
                    for kt in range(4):
                        off = 128 * kt
                        st = pspool.tile([128, 512], f32, tag=f"C{kt % 2}",
                                         name=f"st_{h}_{kt}")
                        nc.tensor.matmul(
                            st[:, off:],
                            lhsT=kt_bf[h][:, kt * 128:(kt + 1) * 128],
                            rhs=qt_bf[h][:, off:])
                        p = pbf_pool.tile([128, 512], fp16, tag="p",
                                          name=f"p_{h}_{kt}")
                        nc.scalar.activation(p[:, off:], st[:, off:],
                                             Exp, scale=SCALE)
                        nc.vector.tensor_mul(p[:, off:off + 128],
                                             p[:, off:off + 128], tri[:])
                        nc.tensor.matmul(rb_ps[:, off:], lhsT=ones16[:],
                                         rhs=p[:, off:],
                                         start=(kt == 0), stop=(kt == 3))
                        nc.tensor.matmul(
                            ctx_ps[:, off:],
                            lhsT=v_bf[:, kt, h * HD:(h + 1) * HD],
                            rhs=p[:, off:], start=(kt == 0),
                            stop=(kt == 3))
                        yield
                    rb = rpool.tile([128, 512], f32, tag="rb", name=f"rb{h}")
                    nc.vector.reciprocal_approx_fast(rb[:], rb_ps[:])
                    nc.vector.tensor_mul(ctx_bf[h][:], ctx_ps[:], rb[:])
                    yield

            def attn_f8(qb):
                """Causal attention for query block qb in 1..3: fp8 q/k/v/P;
                PV and softmax sums as DoubleRow over key-tile pairs."""
                qoff = (qb - 1) * 512
                npair = 2 * qb + 2
                for h in range(HPC):
                    ctx_ps = pspool.tile([128, 512], f32, tag="A",
                                         name=f"actx8_{h}_{qb}")
                    rb_ps = pspool.tile([128, 512], f32, tag="R",
                                        name=f"arb8_{h}_{qb}")
                    for i in range(npair):
                        lo = 0 if i <= 2 * qb else 256
                        p8 = p8_pool.tile([128, 2, 512], fp8, tag="p8",
                                          name=f"p8_{h}_{qb}_{i}")
                        for j in range(2):
                            kt = 2 * i + j
                            st = pspool.tile([128, 512], f32, tag=f"C{j}",
                                             name=f"st8_{h}_{qb}_{kt}")
                            nc.tensor.matmul(
                                st[:, lo:],
                                lhsT=kt_f8[h][:, kt * 128:(kt + 1) * 128],
                                rhs=qt_f8[h][:, qoff + lo:qoff + 512])
                            nc.scalar.activation(p8[:, j, lo:],
                                                 st[:, lo:], Exp,
                                                 scale=SCALE, bias=ebias[:])
                        if i == 2 * qb:
                            nc.vector.tensor_mul(p8[:, 0, 0:128],
                                                 p8[:, 0, 0:128], tri[:])
                            nc.vector.tensor_mul(p8[:, 1, 0:256],
                                                 p8[:, 1, 0:256], ztri[:])
                        elif i == 2 * qb + 1:
                            nc.vector.tensor_mul(p8[:, 0, 256:384],
                                                 p8[:, 0, 256:384],
                                                 tri[:])
                            nc.vector.tensor_mul(p8[:, 1, 256:512],
                                                 p8[:, 1, 256:512],
                                                 ztri[:])
                        nc.tensor.matmul(rb_ps[:, lo:], lhsT=ones8[:],
                                         rhs=p8[:, :, lo:], start=(i == 0),
                                         stop=(i == npair - 1), perf_mode=DR)
                        nc.tensor.matmul(
                            ctx_ps[:, lo:],
                            lhsT=v_f8[:, 2 * i:2 * i + 2,
                                      h * HD:(h + 1) * HD],
                            rhs=p8[:, :, lo:], start=(i == 0),
                            stop=(i == npair - 1), perf_mode=DR)
                        yield
                    rb = rpool.tile([128, 512], f32, tag="rb",
                                    name=f"rb8_{h}_{qb}")
                    nc.vector.reciprocal_approx_fast(rb[:], rb_ps[:])
                    nc.vector.tensor_mul(ctx_f8[:, h, qoff:qoff + 512],
                                         ctx_ps[:], rb[:])
                    yield

            def wo_bf():
                """Output projection for query rows 0-511, bf16; result is
                DMA'd straight from PSUM."""
                for qt in range(4):
                    for db in range(4):
                        ps = pspool.tile([128, 512], f32, tag=f"D{db % 2}",
                                         name=f"o0_{qt}_{db}")
                        for h in range(HPC):
                            nc.tensor.matmul(
                                ps[:],
                                lhsT=ctx_bf[h][:, qt * 128:(qt + 1) * 128],
                                rhs=wo_bf_sb[:, h, db * 512:(db + 1) * 512],
                                start=(h == 0), stop=(h == HPC - 1))
                        osb = opool.tile([128, 512], bf16, tag="o",
                                         name=f"ob_{qt}_{db}")
                        nc.vector.tensor_copy(osb[:], ps[:])
                        eng = nc.sync if db % 2 == 0 else nc.gpsimd
                        eng.dma_start(
                            outp[qt * 128:(qt + 1) * 128,
                                 db * 512:(db + 1) * 512], osb[:])
                        if db % 2 == 1:
                            yield

            def wo_f8(qb):
                """Output projection for query block qb in 1..3, DoubleRow
                over head pairs; result DMA'd straight from PSUM."""
                qoff = (qb - 1) * 512
                for qt in range(4):
                    row = qb * 512 + qt * 128
                    for db in range(4):
                        ps = pspool.tile([128, 512], f32, tag=f"D{db % 2}",
                                         name=f"o{qb}_{qt}_{db}")
                        for j in range(2):
                            nc.tensor.matmul(
                                ps[:],
                                lhsT=ctx_f8[:, 2 * j:2 * j + 2,
                                            qoff + qt * 128:
                                            qoff + (qt + 1) * 128],
                                rhs=wo_f8_sb[:, 2 * j:2 * j + 2,
                                             db * 512:(db + 1) * 512],
                                start=(j == 0), stop=(j == 1), perf_mode=DR)
                        osb = opool.tile([128, 512], bf16, tag="o",
                                         name=f"o8_{qb}_{qt}_{db}")
                        nc.vector.tensor_copy(osb[:], ps[:])
                        eng = nc.sync if db % 2 == 0 else nc.gpsimd
                        eng.dma_start(
                            outp[row:row + 128, db * 512:(db + 1) * 512],
                            osb[:])
                        if db % 2 == 1:
                            yield

            # ---- schedule: software pipeline in program order ----
            _drain(kv_f8(0))
            _weave(kv_f8(1), q_f8(1))
            _weave(attn_f8(1), kv_f8(2), q_f8(2))
            _weave(attn_f8(2), kv_f8(3), q_f8(3), q0_bf())
            _weave(attn_f8(3), kv0_bf())
            _weave(attn_bf(), wo_f8(1), wo_f8(2))
            _weave(wo_bf(), wo_f8(3))

    nc.compile()
    return nc


def _host_constants():
    # sin/cos tables exactly as the flaxformer reference (fp32 math)
    fraction = np.arange(0, HD, 2, dtype=np.float32) / np.float32(HD)
    timescale = (np.float32(10000.0) ** fraction).astype(np.float32)
    sinusoid = np.einsum(
        "i,j->ij", np.arange(S, dtype=np.float32),
        (np.float32(1.0) / timescale)).astype(np.float32)
    sinusoid = np.concatenate([sinusoid, sinusoid], axis=-1)  # [S, HD]
    sinT = np.sin(sinusoid).astype(np.float32).T.copy()
    cosT = np.cos(sinusoid).astype(np.float32).T.copy()
    # rotate_half sign fold: rot(x) rows 0-63 must contribute -x2*sin; the
    # DMA swap moves magnitudes only, so the sign lives in sinN rows 0-63.
    sinN = sinT.copy()
    sinN[:64] *= -1.0

    r = np.arange(128)[:, None]
    c = np.arange(128)[None, :]
    tri = (c - r >= 0).astype(np.float16)
    ztri = np.concatenate([np.zeros((128, 128), np.float16), tri], axis=1)

    return {
        "sinN": sinN.astype(BF16), "cosT": cosT.astype(BF16),
        "ones16": np.ones((128, 128), np.float16),
        "ones8": np.ones((128, 2, 128), np.float32).astype(F8),
        "tri": tri, "ztri": ztri,
    }


def kernel(inputs_q, inputs_kv, wq, wk, wv, wo, mask=None):
    _install_ntff_hook()
    from concourse import bass_utils

    if "nc" not in _CACHE:
        _CACHE["nc"] = _build()
        _CACHE["consts"] = _host_constants()
    nc = _CACHE["nc"]
    consts = _CACHE["consts"]

    wq2 = np.asarray(wq, np.float32).reshape(D, H * HD)
    wk2 = np.asarray(wk, np.float32).reshape(D, H * HD)
    wv2 = np.asarray(wv, np.float32).reshape(D, H * HD)
    wo2 = np.asarray(wo, np.float32).reshape(H * HD, D)
    xq = np.asarray(inputs_q, np.float32)
    xkv = np.asarray(inputs_kv, np.float32)

    in_maps = []
    for cidx in range(N_CORES):
        b, hg = divmod(cidx, H // HPC)
        hs = slice(hg * HW, (hg + 1) * HW)
        xqT = np.ascontiguousarray(xq[b].T)
        xkvT = np.ascontiguousarray(xkv[b].T)
        in_maps.append({
            "xq_bf": xqT[:, :512].astype(BF16),
            "xq_f8": xqT[:, 512:].astype(F8),
            "xkv_bf": xkvT[:, :512].astype(BF16),
            "xkv_f8": xkvT.astype(F8),
            "wq_bf": wq2[:, hs].astype(BF16),
            "wq_f8": wq2[:, hs].astype(F8),
            "wk_bf": wk2[:, hs].astype(BF16),
            "wk_f8": wk2[:, hs].astype(F8),
            "wv_bf": wv2[:, hs].astype(BF16),
            "wv_f8": wv2[:, hs].astype(F8),
            "wo_bf": wo2[hs, :].astype(BF16),
            "wo_f8": wo2[hs, :].astype(F8),
            **consts,
        })

    trace = bool(int(os.environ.get("KERNEL_TRACE", "0")))
    res = bass_utils.run_bass_kernel_spmd(
        nc, in_maps, core_ids=list(range(N_CORES)), trace=trace)
    _CACHE["last_result"] = res

    out = np.zeros((B, S, D), np.float32)
    for cidx in range(N_CORES):
        out[cidx // (H // HPC)] += res.results[cidx]["outp"].astype(np.float32)
    return out


# revision 10
# speedup vs baseline: 1.1646x; 1.0279x over previous
"""Multi-head dot-product attention (B=2, S=2048, D=2048, H=16, HD=128) with
RoPE + causal mask, sharded over 8 NeuronCores: batch (2) x head-groups (4).

Each core computes 4 heads of one batch element end-to-end; the host sums the
four head-group partials per batch element.

Mixed precision: all K/V (and queries >= 512) run fp8 e4m3 with DoubleRow
matmuls (2 contraction chunks per instruction) for the QKV projections, PV,
softmax sums, and the output projection; query rows 0-511 additionally run a
bf16 path end-to-end (bf16 K/V/logits/WO for those rows) so the large-
magnitude early-token outputs stay accurate. exp on the fp8 path carries a -2
bias so probabilities stay inside e4m3 range (normalization cancels it).
rotate_half is done by two SBUF->SBUF partition-swap DMAs with the sign
folded into the sin table. Softmax sums accumulate on the tensor engine via
an all-ones lhsT matmul into a broadcast PSUM tile. Validated vs the fp32
reference: rel err ~1e-2 (budget 2e-2).

Self-contained: hardcodes all shapes; builds/compiles the Bass program once
per process and runs it via run_bass_kernel_spmd on cores 0-7.
"""

import os
import sys
import types

import ml_dtypes
import numpy as np

B, S, D, H, HD = 2, 2048, 2048, 16, 128
HPC = 4                 # heads per core
HW = HPC * HD           # 512: per-core projection width
NDC = D // 128          # 16 contraction chunks of 128
NPR = NDC // 2          # 8 fp8 DoubleRow chunk-pairs
N_CORES = 8
SCALE = float(HD) ** -0.5
EXPB = -2.0             # exp bias on the fp8 path (e4m3 max normal is 240)

BF16 = ml_dtypes.bfloat16
F8 = ml_dtypes.float8_e4m3

_CACHE = {}


def _install_ntff_hook():
    """The image's antenv lacks axon_hooks, so boot() couldn't register the
    NTFF profile hook; recreate the module + hook so trace=True works."""
    if "antenv.axon_hooks" in sys.modules:
        return
    try:
        import antenv  # noqa: F401
        mod = types.ModuleType("antenv.axon_hooks")
        _h = [None]
        mod.set_axon_ntff_profile_hook = lambda h: _h.__setitem__(0, h)
        mod.get_axon_ntff_profile_hook = lambda: _h[0]
        sys.modules["antenv.axon_hooks"] = mod
        from trn_agent_boot.trn_boot import _ntff_profile_via_ctypes
        mod.set_axon_ntff_profile_hook(
            _ntff_profile_via_ctypes("/opt/axon/libaxon_pjrt.so"))
    except Exception:
        pass


def _weave(*streams):
    """Round-robin the generators until all are exhausted: interleaves their
    instruction issue so per-engine queues stay balanced in program order."""
    live = list(streams)
    while live:
        nxt = []
        for g in live:
            try:
                next(g)
                nxt.append(g)
            except StopIteration:
                pass
        live = nxt


def _drain(g):
    for _ in g:
        pass


def _build():
    import concourse.mybir as mybir
    import concourse.tile as tile
    from concourse import bacc

    f32 = mybir.dt.float32
    bf16 = mybir.dt.bfloat16
    fp16 = mybir.dt.float16
    fp8 = mybir.dt.float8e4
    Exp = mybir.ActivationFunctionType.Exp
    DR = mybir.MatmulPerfMode.DoubleRow

    nc = bacc.Bacc("TRN2", target_bir_lowering=False, debug=False,
                   enable_asserts=True, num_devices=N_CORES)

    dram = {}
    for name, shape, dt in [
        ("xq_bf", [D, 512], bf16), ("xq_f8", [D, 1536], fp8),
        ("xkv_bf", [D, 512], bf16), ("xkv_f8", [D, S], fp8),
        ("wq_bf", [D, HW], bf16), ("wq_f8", [D, HW], fp8),
        ("wk_bf", [D, HW], bf16), ("wk_f8", [D, HW], fp8),
        ("wv_bf", [D, HW], bf16), ("wv_f8", [D, HW], fp8),
        ("wo_bf", [HW, D], bf16), ("wo_f8", [HW, D], fp8),
        ("sinN", [HD, S], bf16),    # sin with rows 0-63 negated (rot fold)
        ("cosT", [HD, S], bf16),
        ("ones16", [128, 128], fp16),
        ("ones8", [128, 2, 128], fp8),
        ("tri", [128, 128], fp16), ("ztri", [128, 256], fp16),
    ]:
        dram[name] = nc.dram_tensor(name, shape, dt, kind="ExternalInput").ap()
    outp = nc.dram_tensor("outp", [S, D], bf16, kind="ExternalOutput").ap()

    with tile.TileContext(nc) as tc:
        with (
            tc.tile_pool(name="const", bufs=1) as cpool,
            tc.tile_pool(name="wts", bufs=1) as wpool,
            tc.tile_pool(name="kq", bufs=1) as kq_pool,
            tc.tile_pool(name="vsb", bufs=1) as v_pool,
            tc.tile_pool(name="ctxn", bufs=1) as ctx_pool,
            tc.tile_pool(name="xbf", bufs=1) as xbf_pool,
            tc.tile_pool(name="xf8", bufs=2) as xf8_pool,
            tc.tile_pool(name="raw", bufs=3) as raw_pool,
            tc.tile_pool(name="rot", bufs=3) as rot_pool,
            tc.tile_pool(name="t12", bufs=2) as t12_pool,
            tc.tile_pool(name="pbf", bufs=3) as pbf_pool,
            tc.tile_pool(name="p8", bufs=4) as p8_pool,
            tc.tile_pool(name="rcp", bufs=2) as rpool,
            tc.tile_pool(name="osb", bufs=4) as opool,
            tc.tile_pool(name="ps", space="PSUM", bufs=1) as pspool,
        ):
            def loadw(name, nch, width, dt, eng, ndma):
                """[nch*128, width] dram -> [128, nch, width] sbuf tile."""
                t = wpool.tile([128, nch, width], dt, tag=name, name=name)
                dv = dram[name].rearrange("(n p) w -> p n w", p=128)
                step = nch // ndma
                for i in range(0, nch, step):
                    eng.dma_start(t[:, i:i + step, :], dv[:, i:i + step, :])
                return t

            def loadc(name, shape, dt, eng):
                t = cpool.tile(shape, dt, tag=name, name=name)
                eng.dma_start(t[:], dram[name][:])
                return t

            # ---- startup DMAs: fp8 K/V weights first so compute starts
            # immediately; bf16 weights stream in behind them ----
            wk_f8 = loadw("wk_f8", NDC, HW, fp8, nc.gpsimd, 2)
            wv_f8 = loadw("wv_f8", NDC, HW, fp8, nc.gpsimd, 2)
            sinN = loadc("sinN", [HD, S], bf16, nc.scalar)
            cosT = loadc("cosT", [HD, S], bf16, nc.scalar)
            wq_f8 = loadw("wq_f8", NDC, HW, fp8, nc.gpsimd, 2)
            ones16 = loadc("ones16", [128, 128], fp16, nc.gpsimd)
            ones8 = loadc("ones8", [128, 2, 128], fp8, nc.gpsimd)
            tri = loadc("tri", [128, 128], fp16, nc.gpsimd)
            ztri = loadc("ztri", [128, 256], fp16, nc.gpsimd)
            ebias = cpool.tile([128, 1], f32, tag="ebias", name="ebias")
            nc.gpsimd.memset(ebias[:], EXPB)
            wo_f8_sb = cpool.tile([128, HPC, D], fp8, tag="wo_f8",
                                  name="wo_f8")
            dv = dram["wo_f8"].rearrange("(n p) d -> p n d", p=128)
            nc.gpsimd.dma_start(wo_f8_sb[:], dv[:])
            # bf16 weights load lazily (late_load streams below) so their
            # multi-us DMA issues don't block startup queues
            wts = {}

            def late_load(*names):
                for name in names:
                    if name == "wo_bf":
                        t = cpool.tile([128, HPC, D], bf16, tag="wo_bf",
                                       name="wo_bf")
                        dv = dram["wo_bf"].rearrange("(n p) d -> p n d",
                                                     p=128)
                        nc.sync.dma_start(t[:], dv[:])
                    else:
                        t = loadw(name, NDC, HW, bf16, nc.sync, 1)
                    wts[name] = t
                    yield

            # persistent per-head projection outputs
            kt_bf = [kq_pool.tile([128, 512], bf16, tag=f"ktb{h}",
                                  name=f"ktb{h}") for h in range(HPC)]
            kt_f8 = [kq_pool.tile([128, S], fp8, tag=f"kt8{h}",
                                  name=f"kt8{h}") for h in range(HPC)]
            qt_bf = [kq_pool.tile([128, 512], bf16, tag=f"qtb{h}",
                                  name=f"qtb{h}") for h in range(HPC)]
            qt_f8 = [kq_pool.tile([128, 1536], fp8, tag=f"qt8{h}",
                                  name=f"qt8{h}") for h in range(HPC)]
            v_bf = v_pool.tile([128, 4, 512], fp16, tag="vbf", name="v_bf")
            v_f8 = v_pool.tile([128, 16, 512], fp8, tag="vf8", name="v_f8")
            ctx_bf = [ctx_pool.tile([128, 512], bf16, tag=f"cxb{h}",
                                    name=f"cxb{h}") for h in range(HPC)]
            ctx_f8 = ctx_pool.tile([128, HPC, 1536], fp8, tag="cx8",
                                   name="ctx_f8")

            rot_dma = [nc.gpsimd, nc.gpsimd]

            def rope(ps_acc, tq, out_ap, ridx):
                """out = raw*cos + rot(raw)*sinN for token quarter tq.
                rotate_half = partition swap via 2 SBUF->SBUF DMAs (the sign
                of the top half is folded into sinN)."""
                sl = slice(tq * 512, (tq + 1) * 512)
                raw = raw_pool.tile([128, 512], bf16, tag="raw", name="raw")
                nc.scalar.copy(raw[:], ps_acc[:])
                rot = rot_pool.tile([128, 512], bf16, tag="rot", name="rot")
                eng = rot_dma[ridx % 2]
                eng.dma_start(rot[0:64, :], raw[64:128, :])
                eng.dma_start(rot[64:128, :], raw[0:64, :])
                t1 = t12_pool.tile([128, 512], bf16, tag="t1", name="t1")
                nc.vector.tensor_mul(t1[:], rot[:], sinN[:, sl])
                t2 = t12_pool.tile([128, 512], bf16, tag="t2", name="t2")
                nc.vector.tensor_mul(t2[:], raw[:], cosT[:, sl])
                nc.vector.tensor_add(out_ap, t1[:], t2[:])

            def load_x_bf(dname):
                xt = xbf_pool.tile([128, NDC, 512], bf16, tag="xbf",
                                   name=dname)
                xv = dram[dname].rearrange("(n p) s -> p n s", p=128)
                for c in range(0, NDC, 4):
                    nc.sync.dma_start(xt[:, c:c + 4, :], xv[:, c:c + 4, :])
                return xt

            def load_x_f8(dname, col0):
                xt = xf8_pool.tile([128, NDC, 512], fp8, tag="xf8",
                                   name=f"{dname}_{col0}")
                xv = dram[dname].rearrange("(n p) s -> p n s", p=128)
                sl = slice(col0, col0 + 512)
                for c in range(0, NDC, 8):
                    nc.sync.dma_start(xt[:, c:c + 8, :], xv[:, c:c + 8, sl])
                return xt

            def kv_f8(tq):
                """K+V projection of token quarter tq (0..3), fp8 DoubleRow."""
                xt = load_x_f8("xkv_f8", tq * 512)
                for h in range(HPC):
                    ps = pspool.tile([128, 512], f32, tag=f"B{h % 2}",
                                     name=f"k{tq}_{h}")
                    for c in range(NPR):
                        nc.tensor.matmul(
                            ps[:],
                            lhsT=wk_f8[:, 2 * c:2 * c + 2,
                                       h * HD:(h + 1) * HD],
                            rhs=xt[:, 2 * c:2 * c + 2, :], start=(c == 0),
                            stop=(c == NPR - 1), perf_mode=DR)
                    rope(ps, tq, kt_f8[h][:, tq * 512:(tq + 1) * 512], h)
                    yield
                for ti in range(4):
                    ps = pspool.tile([128, 512], f32, tag=f"D{ti % 2}",
                                     name=f"v{tq}_{ti}")
                    for c in range(NPR):
                        nc.tensor.matmul(
                            ps[:],
                            lhsT=xt[:, 2 * c:2 * c + 2,
                                    ti * 128:(ti + 1) * 128],
                            rhs=wv_f8[:, 2 * c:2 * c + 2, :], start=(c == 0),
                            stop=(c == NPR - 1), perf_mode=DR)
                    nc.scalar.copy(v_f8[:, tq * 4 + ti, :], ps[:])
                    yield

            def kv0_bf():
                """bf16 K+V projection of quarter 0 (for query rows 0-511)."""
                xt = load_x_bf("xkv_bf")
                for h in range(HPC):
                    ps = pspool.tile([128, 512], f32, tag=f"B{h % 2}",
                                     name=f"k0b_{h}")
                    for c in range(NDC):
                        nc.tensor.matmul(
                            ps[:], lhsT=wts["wk_bf"][:, c, h * HD:(h + 1) * HD],
                            rhs=xt[:, c, :], start=(c == 0),
                            stop=(c == NDC - 1))
                    rope(ps, 0, kt_bf[h][:], h)
                    yield
                for ti in range(4):
                    ps = pspool.tile([128, 512], f32, tag=f"D{ti % 2}",
                                     name=f"v0b_{ti}")
                    for c in range(NDC):
                        nc.tensor.matmul(
                            ps[:], lhsT=xt[:, c, ti * 128:(ti + 1) * 128],
                            rhs=wts["wv_bf"][:, c, :], start=(c == 0),
                            stop=(c == NDC - 1))
                    nc.scalar.copy(v_bf[:, ti, :], ps[:])
                    yield

            def q_f8(tq):
                """fp8 Q projection of quarter tq in 1..3."""
                xt = load_x_f8("xq_f8", (tq - 1) * 512)
                for h in range(HPC):
                    ps = pspool.tile([128, 512], f32, tag=f"B{h % 2}",
                                     name=f"q{tq}_{h}")
                    for c in range(NPR):
                        nc.tensor.matmul(
                            ps[:],
                            lhsT=wq_f8[:, 2 * c:2 * c + 2,
                                       h * HD:(h + 1) * HD],
                            rhs=xt[:, 2 * c:2 * c + 2, :], start=(c == 0),
                            stop=(c == NPR - 1), perf_mode=DR)
                    rope(ps, tq,
                         qt_f8[h][:, (tq - 1) * 512:tq * 512], h)
                    yield

            def q0_bf():
                xt = load_x_bf("xq_bf")
                for h in range(HPC):
                    ps = pspool.tile([128, 512], f32, tag=f"B{h % 2}",
                                     name=f"q0b_{h}")
                    for c in range(NDC):
                        nc.tensor.matmul(
                            ps[:], lhsT=wts["wq_bf"][:, c, h * HD:(h + 1) * HD],
                            rhs=xt[:, c, :], start=(c == 0),
                            stop=(c == NDC - 1))
                    rope(ps, 0, qt_bf[h][:], h)
                    yield

            def attn_bf():
                """Causal attention for query rows 0-511, bf16/fp16; softmax
                sum accumulates on the PE via an all-ones lhsT matmul."""
                for h in range(HPC):
                    ctx_ps = pspool.tile([128, 512], f32, tag="A",
                                         name=f"actx_{h}")
                    rb_ps = pspool.tile([128, 512], f32, tag="R",
                                        name=f"arb_{h}")
                    for kt in range(4):
                        off = 128 * kt
                        st = pspool.tile([128, 512], f32, tag=f"C{kt % 2}",
                                         name=f"st_{h}_{kt}")
                        nc.tensor.matmul(
                            st[:, off:],
                            lhsT=kt_bf[h][:, kt * 128:(kt + 1) * 128],
                            rhs=qt_bf[h][:, off:])
                        p = pbf_pool.tile([128, 512], fp16, tag="p",
                                          name=f"p_{h}_{kt}")
                        nc.scalar.activation(p[:, off:], st[:, off:],
                                             Exp, scale=SCALE)
                        nc.vector.tensor_mul(p[:, off:off + 128],
                                             p[:, off:off + 128], tri[:])
                        nc.tensor.matmul(rb_ps[:, off:], lhsT=ones16[:],
                                         rhs=p[:, off:],
                                         start=(kt == 0), stop=(kt == 3))
                        nc.tensor.matmul(
                            ctx_ps[:, off:],
                            lhsT=v_bf[:, kt, h * HD:(h + 1) * HD],
                            rhs=p[:, off:], start=(kt == 0),
                            stop=(kt == 3))
                        yield
                    rb = rpool.tile([128, 512], f32, tag="rb", name=f"rb{h}")
                    nc.vector.reciprocal_approx_fast(rb[:], rb_ps[:])
                    nc.vector.tensor_mul(ctx_bf[h][:], ctx_ps[:], rb[:])
                    yield

            def attn_f8(qb):
                """Causal attention for query block qb in 1..3: fp8 q/k/v/P;
                PV and softmax sums as DoubleRow over key-tile pairs."""
                qoff = (qb - 1) * 512
                npair = 2 * qb + 2
                for h in range(HPC):
                    ctx_ps = pspool.tile([128, 512], f32, tag="A",
                                         name=f"actx8_{h}_{qb}")
                    rb_ps = pspool.tile([128, 512], f32, tag="R",
                                        name=f"arb8_{h}_{qb}")
                    for i in range(npair):
                        lo = 0 if i <= 2 * qb else 256
                        p8 = p8_pool.tile([128, 2, 512], fp8, tag="p8",
                                          name=f"p8_{h}_{qb}_{i}")
                        for j in range(2):
                            kt = 2 * i + j
                            st = pspool.tile([128, 512], f32, tag=f"C{j}",
                                             name=f"st8_{h}_{qb}_{kt}")
                            nc.tensor.matmul(
                                st[:, lo:],
                                lhsT=kt_f8[h][:, kt * 128:(kt + 1) * 128],
                                rhs=qt_f8[h][:, qoff + lo:qoff + 512])
                            nc.scalar.activation(p8[:, j, lo:],
                                                 st[:, lo:], Exp,
                                                 scale=SCALE, bias=ebias[:])
                        if i == 2 * qb:
                            nc.vector.tensor_mul(p8[:, 0, 0:128],
                                                 p8[:, 0, 0:128], tri[:])
                            nc.vector.tensor_mul(p8[:, 1, 0:256],
                                                 p8[:, 1, 0:256], ztri[:])
                        elif i == 2 * qb + 1:
                            nc.vector.tensor_mul(p8[:, 0, 256:384],
                                                 p8[:, 0, 256:384],
                                                 tri[:])
                            nc.vector.tensor_mul(p8[:, 1, 256:512],
                                                 p8[:, 1, 256:512],
                                                 ztri[:])
                        nc.tensor.matmul(rb_ps[:, lo:], lhsT=ones8[:],
                                         rhs=p8[:, :, lo:], start=(i == 0),
                                         stop=(i == npair - 1), perf_mode=DR)
                        nc.tensor.matmul(
                            ctx_ps[:, lo:],
                            lhsT=v_f8[:, 2 * i:2 * i + 2,
                                      h * HD:(h + 1) * HD],
                            rhs=p8[:, :, lo:], start=(i == 0),
                            stop=(i == npair - 1), perf_mode=DR)
                        yield
                    rb = rpool.tile([128, 512], f32, tag="rb",
                                    name=f"rb8_{h}_{qb}")
                    nc.vector.reciprocal_approx_fast(rb[:], rb_ps[:])
                    nc.vector.tensor_mul(ctx_f8[:, h, qoff:qoff + 512],
                                         ctx_ps[:], rb[:])
                    yield

            def wo_bf():
                """Output projection for query rows 0-511, bf16; result is
                DMA'd straight from PSUM."""
                for qt in range(4):
                    for db in range(4):
                        ps = pspool.tile([128, 512], f32, tag=f"D{db % 2}",
                                         name=f"o0_{qt}_{db}")
                        for h in range(HPC):
                            nc.tensor.matmul(
                                ps[:],
                                lhsT=ctx_bf[h][:, qt * 128:(qt + 1) * 128],
                                rhs=wts["wo_bf"][:, h, db * 512:(db + 1) * 512],
                                start=(h == 0), stop=(h == HPC - 1))
                        osb = opool.tile([128, 512], bf16, tag="o",
                                         name=f"ob_{qt}_{db}")
                        nc.vector.tensor_copy(osb[:], ps[:])
                        eng = nc.sync if db % 2 == 0 else nc.gpsimd
                        eng.dma_start(
                            outp[qt * 128:(qt + 1) * 128,
                                 db * 512:(db + 1) * 512], osb[:])
                        if db % 2 == 1:
                            yield

            def wo_f8(qb):
                """Output projection for query block qb in 1..3, DoubleRow
                over head pairs; result DMA'd straight from PSUM."""
                qoff = (qb - 1) * 512
                for qt in range(4):
                    row = qb * 512 + qt * 128
                    for db in range(4):
                        ps = pspool.tile([128, 512], f32, tag=f"D{db % 2}",
                                         name=f"o{qb}_{qt}_{db}")
                        for j in range(2):
                            nc.tensor.matmul(
                                ps[:],
                                lhsT=ctx_f8[:, 2 * j:2 * j + 2,
                                            qoff + qt * 128:
                                            qoff + (qt + 1) * 128],
                                rhs=wo_f8_sb[:, 2 * j:2 * j + 2,
                                             db * 512:(db + 1) * 512],
                                start=(j == 0), stop=(j == 1), perf_mode=DR)
                        osb = opool.tile([128, 512], bf16, tag="o",
                                         name=f"o8_{qb}_{qt}_{db}")
                        nc.vector.tensor_copy(osb[:], ps[:])
                        eng = nc.sync if db % 2 == 0 else nc.gpsimd
                        eng.dma_start(
                            outp[row:row + 128, db * 512:(db + 1) * 512],
                            osb[:])
                        if db % 2 == 1:
                            yield

            # ---- schedule: software pipeline in program order ----
            _drain(kv_f8(0))
            _weave(kv_f8(1), q_f8(1), late_load("wq_bf"))
            _weave(attn_f8(1), kv_f8(2), q_f8(2),
                   late_load("wk_bf", "wv_bf"))
            _weave(attn_f8(2), kv_f8(3), q_f8(3), q0_bf(),
                   late_load("wo_bf"))
            _weave(attn_f8(3), kv0_bf())
            _weave(attn_bf(), wo_f8(1), wo_f8(2))
            _weave(wo_bf(), wo_f8(3))

    nc.compile()
    return nc


def _host_constants():
    # sin/cos tables exactly as the flaxformer reference (fp32 math)
    fraction = np.arange(0, HD, 2, dtype=np.float32) / np.float32(HD)
    timescale = (np.float32(10000.0) ** fraction).astype(np.float32)
    sinusoid = np.einsum(
        "i,j->ij", np.arange(S, dtype=np.float32),
        (np.float32(1.0) / timescale)).astype(np.float32)
    sinusoid = np.concatenate([sinusoid, sinusoid], axis=-1)  # [S, HD]
    sinT = np.sin(sinusoid).astype(np.float32).T.copy()
    cosT = np.cos(sinusoid).astype(np.float32).T.copy()
    # rotate_half sign fold: rot(x) rows 0-63 must contribute -x2*sin; the
    # DMA swap moves magnitudes only, so the sign lives in sinN rows 0-63.
    sinN = sinT.copy()
    sinN[:64] *= -1.0

    r = np.arange(128)[:, None]
    c = np.arange(128)[None, :]
    tri = (c - r >= 0).astype(np.float16)
    ztri = np.concatenate([np.zeros((128, 128), np.float16), tri], axis=1)

    return {
        "sinN": sinN.astype(BF16), "cosT": cosT.astype(BF16),
        "ones16": np.ones((128, 128), np.float16),
        "ones8": np.ones((128, 2, 128), np.float32).astype(F8),
        "tri": tri, "ztri": ztri,
    }


def kernel(inputs_q, inputs_kv, wq, wk, wv, wo, mask=None):
    _install_ntff_hook()
    from concourse import bass_utils

    if "nc" not in _CACHE:
        _CACHE["nc"] = _build()
        _CACHE["consts"] = _host_constants()
    nc = _CACHE["nc"]
    consts = _CACHE["consts"]

    wq2 = np.asarray(wq, np.float32).reshape(D, H * HD)
    wk2 = np.asarray(wk, np.float32).reshape(D, H * HD)
    wv2 = np.asarray(wv, np.float32).reshape(D, H * HD)
    wo2 = np.asarray(wo, np.float32).reshape(H * HD, D)
    xq = np.asarray(inputs_q, np.float32)
    xkv = np.asarray(inputs_kv, np.float32)

    in_maps = []
    for cidx in range(N_CORES):
        b, hg = divmod(cidx, H // HPC)
        hs = slice(hg * HW, (hg + 1) * HW)
        xqT = np.ascontiguousarray(xq[b].T)
        xkvT = np.ascontiguousarray(xkv[b].T)
        in_maps.append({
            "xq_bf": xqT[:, :512].astype(BF16),
            "xq_f8": xqT[:, 512:].astype(F8),
            "xkv_bf": xkvT[:, :512].astype(BF16),
            "xkv_f8": xkvT.astype(F8),
            "wq_bf": wq2[:, hs].astype(BF16),
            "wq_f8": wq2[:, hs].astype(F8),
            "wk_bf": wk2[:, hs].astype(BF16),
            "wk_f8": wk2[:, hs].astype(F8),
            "wv_bf": wv2[:, hs].astype(BF16),
            "wv_f8": wv2[:, hs].astype(F8),
            "wo_bf": wo2[hs, :].astype(BF16),
            "wo_f8": wo2[hs, :].astype(F8),
            **consts,
        })

    trace = bool(int(os.environ.get("KERNEL_TRACE", "0")))
    res = bass_utils.run_bass_kernel_spmd(
        nc, in_maps, core_ids=list(range(N_CORES)), trace=trace)
    _CACHE["last_result"] = res

    out = np.zeros((B, S, D), np.float32)
    for cidx in range(N_CORES):
        out[cidx // (H // HPC)] += res.results[cidx]["outp"].astype(np.float32)
    return out


# revision 12
# speedup vs baseline: 1.1734x; 1.0075x over previous
"""Multi-head dot-product attention (B=2, S=2048, D=2048, H=16, HD=128) with
RoPE + causal mask, sharded over 8 NeuronCores: batch (2) x head-groups (4).

Each core computes 4 heads of one batch element end-to-end; the host sums the
four head-group partials per batch element.

Mixed precision: all K/V (and queries >= 512) run fp8 e4m3 with DoubleRow
matmuls (2 contraction chunks per instruction) for the QKV projections, PV,
softmax sums, and the output projection; query rows 0-511 additionally run a
bf16 path end-to-end (bf16 K/V/logits/WO for those rows) so the large-
magnitude early-token outputs stay accurate. exp on the fp8 path carries a -2
bias so probabilities stay inside e4m3 range (normalization cancels it).
rotate_half is done by two SBUF->SBUF partition-swap DMAs with the sign
folded into the sin table. Softmax sums accumulate on the tensor engine via
an all-ones lhsT matmul into a broadcast PSUM tile. Validated vs the fp32
reference: rel err ~1e-2 (budget 2e-2).

Self-contained: hardcodes all shapes; builds/compiles the Bass program once
per process and runs it via run_bass_kernel_spmd on cores 0-7.
"""

import os
import sys
import types

import ml_dtypes
import numpy as np

B, S, D, H, HD = 2, 2048, 2048, 16, 128
HPC = 4                 # heads per core
HW = HPC * HD           # 512: per-core projection width
NDC = D // 128          # 16 contraction chunks of 128
NPR = NDC // 2          # 8 fp8 DoubleRow chunk-pairs
N_CORES = 8
SCALE = float(HD) ** -0.5
EXPB = -2.0             # exp bias on the fp8 path (e4m3 max normal is 240)

BF16 = ml_dtypes.bfloat16
F8 = ml_dtypes.float8_e4m3

_CACHE = {}


def _install_ntff_hook():
    """The image's antenv lacks axon_hooks, so boot() couldn't register the
    NTFF profile hook; recreate the module + hook so trace=True works."""
    if "antenv.axon_hooks" in sys.modules:
        return
    try:
        import antenv  # noqa: F401
        mod = types.ModuleType("antenv.axon_hooks")
        _h = [None]
        mod.set_axon_ntff_profile_hook = lambda h: _h.__setitem__(0, h)
        mod.get_axon_ntff_profile_hook = lambda: _h[0]
        sys.modules["antenv.axon_hooks"] = mod
        from trn_agent_boot.trn_boot import _ntff_profile_via_ctypes
        mod.set_axon_ntff_profile_hook(
            _ntff_profile_via_ctypes("/opt/axon/libaxon_pjrt.so"))
    except Exception:
        pass


def _weave(*streams):
    """Round-robin the generators until all are exhausted: interleaves their
    instruction issue so per-engine queues stay balanced in program order."""
    live = list(streams)
    while live:
        nxt = []
        for g in live:
            try:
                next(g)
                nxt.append(g)
            except StopIteration:
                pass
        live = nxt


def _drain(g):
    for _ in g:
        pass


def _build():
    import concourse.mybir as mybir
    import concourse.tile as tile
    from concourse import bacc

    f32 = mybir.dt.float32
    bf16 = mybir.dt.bfloat16
    fp16 = mybir.dt.float16
    fp8 = mybir.dt.float8e4
    Exp = mybir.ActivationFunctionType.Exp
    DR = mybir.MatmulPerfMode.DoubleRow

    nc = bacc.Bacc("TRN2", target_bir_lowering=False, debug=False,
                   enable_asserts=True, num_devices=N_CORES)

    dram = {}
    for name, shape, dt in [
        ("xq_bf", [D, 512], bf16), ("xq_f8", [D, 1536], fp8),
        ("xkv_bf", [D, 512], bf16), ("xkv_f8", [D, S], fp8),
        ("wq_bf", [D, HW], bf16), ("wq_f8", [D, HW], fp8),
        ("wk_bf", [D, HW], bf16), ("wk_f8", [D, HW], fp8),
        ("wv_bf", [D, HW], bf16), ("wv_f8", [D, HW], fp8),
        ("wo_bf", [HW, D], bf16), ("wo_f8", [HW, D], fp8),
        ("sinN", [HD, S], bf16),    # sin with rows 0-63 negated (rot fold)
        ("cosT", [HD, S], bf16),
        ("ones16", [128, 128], fp16),
        ("ones8", [128, 2, 128], fp8),
        ("tri", [128, 128], fp16), ("ztri", [128, 256], fp16),
    ]:
        dram[name] = nc.dram_tensor(name, shape, dt, kind="ExternalInput").ap()
    outp = nc.dram_tensor("outp", [S, D], bf16, kind="ExternalOutput").ap()

    with tile.TileContext(nc) as tc:
        with (
            tc.tile_pool(name="const", bufs=1) as cpool,
            tc.tile_pool(name="wts", bufs=1) as wpool,
            tc.tile_pool(name="kq", bufs=1) as kq_pool,
            tc.tile_pool(name="vsb", bufs=1) as v_pool,
            tc.tile_pool(name="ctxn", bufs=1) as ctx_pool,
            tc.tile_pool(name="xbf", bufs=1) as xbf_pool,
            tc.tile_pool(name="xf8", bufs=2) as xf8_pool,
            tc.tile_pool(name="raw", bufs=3) as raw_pool,
            tc.tile_pool(name="rot", bufs=3) as rot_pool,
            tc.tile_pool(name="t12", bufs=2) as t12_pool,
            tc.tile_pool(name="pbf", bufs=3) as pbf_pool,
            tc.tile_pool(name="p8", bufs=4) as p8_pool,
            tc.tile_pool(name="rcp", bufs=2) as rpool,
            tc.tile_pool(name="osb", bufs=4) as opool,
            tc.tile_pool(name="ps", space="PSUM", bufs=1) as pspool,
        ):
            def loadw(name, nch, width, dt, eng, ndma):
                """[nch*128, width] dram -> [128, nch, width] sbuf tile."""
                t = wpool.tile([128, nch, width], dt, tag=name, name=name)
                dv = dram[name].rearrange("(n p) w -> p n w", p=128)
                step = nch // ndma
                for i in range(0, nch, step):
                    eng.dma_start(t[:, i:i + step, :], dv[:, i:i + step, :])
                return t

            def loadc(name, shape, dt, eng):
                t = cpool.tile(shape, dt, tag=name, name=name)
                eng.dma_start(t[:], dram[name][:])
                return t

            # ---- startup DMAs: fp8 K/V weights first so compute starts
            # immediately; bf16 weights stream in behind them ----
            wk_f8 = loadw("wk_f8", NDC, HW, fp8, nc.gpsimd, 2)
            wv_f8 = loadw("wv_f8", NDC, HW, fp8, nc.gpsimd, 2)
            sinN = loadc("sinN", [HD, S], bf16, nc.scalar)
            cosT = loadc("cosT", [HD, S], bf16, nc.scalar)
            wq_f8 = loadw("wq_f8", NDC, HW, fp8, nc.gpsimd, 2)
            ones16 = loadc("ones16", [128, 128], fp16, nc.gpsimd)
            ones8 = loadc("ones8", [128, 2, 128], fp8, nc.gpsimd)
            tri = loadc("tri", [128, 128], fp16, nc.gpsimd)
            ztri = loadc("ztri", [128, 256], fp16, nc.gpsimd)
            ebias = cpool.tile([128, 1], f32, tag="ebias", name="ebias")
            nc.gpsimd.memset(ebias[:], EXPB)
            wo_f8_sb = cpool.tile([128, HPC, D], fp8, tag="wo_f8",
                                  name="wo_f8")
            dv = dram["wo_f8"].rearrange("(n p) d -> p n d", p=128)
            nc.gpsimd.dma_start(wo_f8_sb[:], dv[:])
            # bf16 weights load lazily (late_load streams below) so their
            # multi-us DMA issues don't block startup queues
            wts = {}

            def late_load(*names):
                for name in names:
                    if name == "wo_bf":
                        t = cpool.tile([128, HPC, D], bf16, tag="wo_bf",
                                       name="wo_bf")
                        dv = dram["wo_bf"].rearrange("(n p) d -> p n d",
                                                     p=128)
                        nc.sync.dma_start(t[:], dv[:])
                    else:
                        t = loadw(name, NDC, HW, bf16, nc.sync, 1)
                    wts[name] = t
                    yield

            # persistent per-head projection outputs
            kt_bf = [kq_pool.tile([128, 512], bf16, tag=f"ktb{h}",
                                  name=f"ktb{h}") for h in range(HPC)]
            kt_f8 = [kq_pool.tile([128, S], fp8, tag=f"kt8{h}",
                                  name=f"kt8{h}") for h in range(HPC)]
            qt_bf = [kq_pool.tile([128, 512], bf16, tag=f"qtb{h}",
                                  name=f"qtb{h}") for h in range(HPC)]
            qt_f8 = [kq_pool.tile([128, 1536], fp8, tag=f"qt8{h}",
                                  name=f"qt8{h}") for h in range(HPC)]
            v_bf = v_pool.tile([128, 4, 512], fp16, tag="vbf", name="v_bf")
            v_f8 = v_pool.tile([128, 16, 512], fp8, tag="vf8", name="v_f8")
            ctx_bf = [ctx_pool.tile([128, 512], bf16, tag=f"cxb{h}",
                                    name=f"cxb{h}") for h in range(HPC)]
            ctx_f8 = ctx_pool.tile([128, HPC, 1536], fp8, tag="cx8",
                                   name="ctx_f8")

            rot_dma = [nc.gpsimd, nc.gpsimd]

            def rope(ps_acc, tq, out_ap, ridx):
                """out = raw*cos + rot(raw)*sinN for token quarter tq.
                rotate_half = partition swap via 2 SBUF->SBUF DMAs (the sign
                of the top half is folded into sinN)."""
                sl = slice(tq * 512, (tq + 1) * 512)
                raw = raw_pool.tile([128, 512], bf16, tag="raw", name="raw")
                nc.scalar.copy(raw[:], ps_acc[:])
                rot = rot_pool.tile([128, 512], bf16, tag="rot", name="rot")
                eng = rot_dma[ridx % 2]
                eng.dma_start(rot[0:64, :], raw[64:128, :])
                eng.dma_start(rot[64:128, :], raw[0:64, :])
                t1 = t12_pool.tile([128, 512], bf16, tag="t1", name="t1")
                nc.vector.tensor_mul(t1[:], rot[:], sinN[:, sl])
                t2 = t12_pool.tile([128, 512], bf16, tag="t2", name="t2")
                nc.vector.tensor_mul(t2[:], raw[:], cosT[:, sl])
                nc.vector.tensor_add(out_ap, t1[:], t2[:])

            def load_x_bf(dname):
                xt = xbf_pool.tile([128, NDC, 512], bf16, tag="xbf",
                                   name=dname)
                xv = dram[dname].rearrange("(n p) s -> p n s", p=128)
                for c in range(0, NDC, 4):
                    nc.sync.dma_start(xt[:, c:c + 4, :], xv[:, c:c + 4, :])
                return xt

            def load_x_f8(dname, col0):
                xt = xf8_pool.tile([128, NDC, 512], fp8, tag="xf8",
                                   name=f"{dname}_{col0}")
                xv = dram[dname].rearrange("(n p) s -> p n s", p=128)
                sl = slice(col0, col0 + 512)
                for c in range(0, NDC, 8):
                    nc.sync.dma_start(xt[:, c:c + 8, :], xv[:, c:c + 8, sl])
                return xt

            def kv_f8(tq):
                """K+V projection of token quarter tq (0..3), fp8 DoubleRow."""
                xt = load_x_f8("xkv_f8", tq * 512)
                for h in range(HPC):
                    ps = pspool.tile([128, 512], f32, tag=f"B{h % 2}",
                                     name=f"k{tq}_{h}")
                    for c in range(NPR):
                        nc.tensor.matmul(
                            ps[:],
                            lhsT=wk_f8[:, 2 * c:2 * c + 2,
                                       h * HD:(h + 1) * HD],
                            rhs=xt[:, 2 * c:2 * c + 2, :], start=(c == 0),
                            stop=(c == NPR - 1), perf_mode=DR)
                    rope(ps, tq, kt_f8[h][:, tq * 512:(tq + 1) * 512], h)
                    yield
                for ti in range(4):
                    ps = pspool.tile([128, 512], f32, tag=f"D{ti % 2}",
                                     name=f"v{tq}_{ti}")
                    for c in range(NPR):
                        nc.tensor.matmul(
                            ps[:],
                            lhsT=xt[:, 2 * c:2 * c + 2,
                                    ti * 128:(ti + 1) * 128],
                            rhs=wv_f8[:, 2 * c:2 * c + 2, :], start=(c == 0),
                            stop=(c == NPR - 1), perf_mode=DR)
                    nc.scalar.copy(v_f8[:, tq * 4 + ti, :], ps[:])
                    yield

            def kv0_bf():
                """bf16 K+V projection of quarter 0 (for query rows 0-511)."""
                xt = load_x_bf("xkv_bf")
                for h in range(HPC):
                    ps = pspool.tile([128, 512], f32, tag=f"B{h % 2}",
                                     name=f"k0b_{h}")
                    for c in range(NDC):
                        nc.tensor.matmul(
                            ps[:], lhsT=wts["wk_bf"][:, c, h * HD:(h + 1) * HD],
                            rhs=xt[:, c, :], start=(c == 0),
                            stop=(c == NDC - 1))
                    rope(ps, 0, kt_bf[h][:], h)
                    yield
                for ti in range(4):
                    ps = pspool.tile([128, 512], f32, tag=f"D{ti % 2}",
                                     name=f"v0b_{ti}")
                    for c in range(NDC):
                        nc.tensor.matmul(
                            ps[:], lhsT=xt[:, c, ti * 128:(ti + 1) * 128],
                            rhs=wts["wv_bf"][:, c, :], start=(c == 0),
                            stop=(c == NDC - 1))
                    nc.scalar.copy(v_bf[:, ti, :], ps[:])
                    yield

            def q_f8(tq):
                """fp8 Q projection of quarter tq in 1..3."""
                xt = load_x_f8("xq_f8", (tq - 1) * 512)
                for h in range(HPC):
                    ps = pspool.tile([128, 512], f32, tag=f"B{h % 2}",
                                     name=f"q{tq}_{h}")
                    for c in range(NPR):
                        nc.tensor.matmul(
                            ps[:],
                            lhsT=wq_f8[:, 2 * c:2 * c + 2,
                                       h * HD:(h + 1) * HD],
                            rhs=xt[:, 2 * c:2 * c + 2, :], start=(c == 0),
                            stop=(c == NPR - 1), perf_mode=DR)
                    rope(ps, tq,
                         qt_f8[h][:, (tq - 1) * 512:tq * 512], h)
                    yield

            def q0_bf():
                xt = load_x_bf("xq_bf")
                for h in range(HPC):
                    ps = pspool.tile([128, 512], f32, tag=f"B{h % 2}",
                                     name=f"q0b_{h}")
                    for c in range(NDC):
                        nc.tensor.matmul(
                            ps[:], lhsT=wts["wq_bf"][:, c, h * HD:(h + 1) * HD],
                            rhs=xt[:, c, :], start=(c == 0),
                            stop=(c == NDC - 1))
                    rope(ps, 0, qt_bf[h][:], h)
                    yield

            def attn_bf():
                """Causal attention for query rows 0-511, bf16/fp16; softmax
                sum accumulates on the PE via an all-ones lhsT matmul."""
                for h in range(HPC):
                    ctx_ps = pspool.tile([128, 512], f32, tag="A",
                                         name=f"actx_{h}")
                    rb_ps = pspool.tile([128, 512], f32, tag="R",
                                        name=f"arb_{h}")
                    for kt in range(4):
                        off = 128 * kt
                        st = pspool.tile([128, 512], f32, tag=f"C{kt % 2}",
                                         name=f"st_{h}_{kt}")
                        nc.tensor.matmul(
                            st[:, off:],
                            lhsT=kt_bf[h][:, kt * 128:(kt + 1) * 128],
                            rhs=qt_bf[h][:, off:])
                        p = pbf_pool.tile([128, 512], fp16, tag="p",
                                          name=f"p_{h}_{kt}")
                        nc.scalar.activation(p[:, off:], st[:, off:],
                                             Exp, scale=SCALE)
                        nc.vector.tensor_mul(p[:, off:off + 128],
                                             p[:, off:off + 128], tri[:])
                        nc.tensor.matmul(rb_ps[:, off:], lhsT=ones16[:],
                                         rhs=p[:, off:],
                                         start=(kt == 0), stop=(kt == 3))
                        nc.tensor.matmul(
                            ctx_ps[:, off:],
                            lhsT=v_bf[:, kt, h * HD:(h + 1) * HD],
                            rhs=p[:, off:], start=(kt == 0),
                            stop=(kt == 3))
                        yield
                    rb = rpool.tile([128, 512], f32, tag="rb", name=f"rb{h}")
                    nc.vector.reciprocal_approx_fast(rb[:], rb_ps[:])
                    nc.vector.tensor_mul(ctx_bf[h][:], ctx_ps[:], rb[:])
                    yield

            def attn_f8(qb):
                """Causal attention for query block qb in 1..3: fp8 q/k/v/P;
                PV and softmax sums as DoubleRow over key-tile pairs."""
                qoff = (qb - 1) * 512
                npair = 2 * qb + 2
                for h in range(HPC):
                    ctx_ps = pspool.tile([128, 512], f32, tag="A",
                                         name=f"actx8_{h}_{qb}")
                    rb_ps = pspool.tile([128, 512], f32, tag="R",
                                        name=f"arb8_{h}_{qb}")
                    for i in range(npair):
                        lo = 0 if i <= 2 * qb else 256
                        p8 = p8_pool.tile([128, 2, 512], fp8, tag="p8",
                                          name=f"p8_{h}_{qb}_{i}")
                        for j in range(2):
                            kt = 2 * i + j
                            st = pspool.tile([128, 512], f32, tag=f"C{j}",
                                             name=f"st8_{h}_{qb}_{kt}")
                            nc.tensor.matmul(
                                st[:, lo:],
                                lhsT=kt_f8[h][:, kt * 128:(kt + 1) * 128],
                                rhs=qt_f8[h][:, qoff + lo:qoff + 512])
                            nc.scalar.activation(p8[:, j, lo:],
                                                 st[:, lo:], Exp,
                                                 scale=SCALE, bias=ebias[:])
                        if i == 2 * qb:
                            nc.vector.tensor_mul(p8[:, 0, 0:128],
                                                 p8[:, 0, 0:128], tri[:])
                            nc.vector.tensor_mul(p8[:, 1, 0:256],
                                                 p8[:, 1, 0:256], ztri[:])
                        elif i == 2 * qb + 1:
                            nc.vector.tensor_mul(p8[:, 0, 256:384],
                                                 p8[:, 0, 256:384],
                                                 tri[:])
                            nc.vector.tensor_mul(p8[:, 1, 256:512],
                                                 p8[:, 1, 256:512],
                                                 ztri[:])
                        nc.tensor.matmul(rb_ps[:, lo:], lhsT=ones8[:],
                                         rhs=p8[:, :, lo:], start=(i == 0),
                                         stop=(i == npair - 1), perf_mode=DR)
                        nc.tensor.matmul(
                            ctx_ps[:, lo:],
                            lhsT=v_f8[:, 2 * i:2 * i + 2,
                                      h * HD:(h + 1) * HD],
                            rhs=p8[:, :, lo:], start=(i == 0),
                            stop=(i == npair - 1), perf_mode=DR)
                        yield
                    rb = rpool.tile([128, 512], f32, tag="rb",
                                    name=f"rb8_{h}_{qb}")
                    nc.vector.reciprocal_approx_fast(rb[:], rb_ps[:])
                    nc.vector.tensor_mul(ctx_f8[:, h, qoff:qoff + 512],
                                         ctx_ps[:], rb[:])
                    yield

            def wo_bf(ceng):
                """Output projection for query rows 0-511, bf16; result is
                DMA'd straight from PSUM."""
                for qt in range(4):
                    for db in range(4):
                        ps = pspool.tile([128, 512], f32, tag=f"D{db % 2}",
                                         name=f"o0_{qt}_{db}")
                        for h in range(HPC):
                            nc.tensor.matmul(
                                ps[:],
                                lhsT=ctx_bf[h][:, qt * 128:(qt + 1) * 128],
                                rhs=wts["wo_bf"][:, h, db * 512:(db + 1) * 512],
                                start=(h == 0), stop=(h == HPC - 1))
                        osb = opool.tile([128, 512], bf16, tag="o",
                                         name=f"ob_{qt}_{db}")
                        ceng(osb[:], ps[:])
                        eng = nc.sync if db % 2 == 0 else nc.gpsimd
                        eng.dma_start(
                            outp[qt * 128:(qt + 1) * 128,
                                 db * 512:(db + 1) * 512], osb[:])
                        if db % 2 == 1:
                            yield

            def wo_f8(qb, ceng):
                """Output projection for query block qb in 1..3, DoubleRow
                over head pairs; result DMA'd straight from PSUM."""
                qoff = (qb - 1) * 512
                for qt in range(4):
                    row = qb * 512 + qt * 128
                    for db in range(4):
                        ps = pspool.tile([128, 512], f32, tag=f"D{db % 2}",
                                         name=f"o{qb}_{qt}_{db}")
                        for j in range(2):
                            nc.tensor.matmul(
                                ps[:],
                                lhsT=ctx_f8[:, 2 * j:2 * j + 2,
                                            qoff + qt * 128:
                                            qoff + (qt + 1) * 128],
                                rhs=wo_f8_sb[:, 2 * j:2 * j + 2,
                                             db * 512:(db + 1) * 512],
                                start=(j == 0), stop=(j == 1), perf_mode=DR)
                        osb = opool.tile([128, 512], bf16, tag="o",
                                         name=f"o8_{qb}_{qt}_{db}")
                        ceng(osb[:], ps[:])
                        eng = nc.sync if db % 2 == 0 else nc.gpsimd
                        eng.dma_start(
                            outp[row:row + 128, db * 512:(db + 1) * 512],
                            osb[:])
                        if db % 2 == 1:
                            yield

            # ---- schedule: software pipeline in program order ----
            _drain(kv_f8(0))
            _weave(kv_f8(1), q_f8(1), late_load("wq_bf"))
            _weave(attn_f8(1), kv_f8(2), q_f8(2),
                   late_load("wk_bf", "wv_bf"))
            _weave(attn_f8(2), kv_f8(3), q_f8(3), q0_bf(),
                   late_load("wo_bf"))
            _weave(attn_f8(3), kv0_bf())
            _weave(attn_bf(), wo_f8(1, nc.vector.tensor_copy),
                   wo_f8(2, nc.scalar.copy))
            _weave(wo_bf(nc.scalar.copy), wo_f8(3, nc.scalar.copy))

    nc.compile()
    return nc


def _host_constants():
    # sin/cos tables exactly as the flaxformer reference (fp32 math)
    fraction = np.arange(0, HD, 2, dtype=np.float32) / np.float32(HD)
    timescale = (np.float32(10000.0) ** fraction).astype(np.float32)
    sinusoid = np.einsum(
        "i,j->ij", np.arange(S, dtype=np.float32),
        (np.float32(1.0) / timescale)).astype(np.float32)
    sinusoid = np.concatenate([sinusoid, sinusoid], axis=-1)  # [S, HD]
    sinT = np.sin(sinusoid).astype(np.float32).T.copy()
    cosT = np.cos(sinusoid).astype(np.float32).T.copy()
    # rotate_half sign fold: rot(x) rows 0-63 must contribute -x2*sin; the
    # DMA swap moves magnitudes only, so the sign lives in sinN rows 0-63.
    sinN = sinT.copy()
    sinN[:64] *= -1.0

    r = np.arange(128)[:, None]
    c = np.arange(128)[None, :]
    tri = (c - r >= 0).astype(np.float16)
    ztri = np.concatenate([np.zeros((128, 128), np.float16), tri], axis=1)

    return {
        "sinN": sinN.astype(BF16), "cosT": cosT.astype(BF16),
        "ones16": np.ones((128, 128), np.float16),
        "ones8": np.ones((128, 2, 128), np.float32).astype(F8),
        "tri": tri, "ztri": ztri,
    }


def kernel(inputs_q, inputs_kv, wq, wk, wv, wo, mask=None):
    _install_ntff_hook()
    from concourse import bass_utils

    if "nc" not in _CACHE:
        _CACHE["nc"] = _build()
        _CACHE["consts"] = _host_constants()
    nc = _CACHE["nc"]
    consts = _CACHE["consts"]

    wq2 = np.asarray(wq, np.float32).reshape(D, H * HD)
    wk2 = np.asarray(wk, np.float32).reshape(D, H * HD)
    wv2 = np.asarray(wv, np.float32).reshape(D, H * HD)
    wo2 = np.asarray(wo, np.float32).reshape(H * HD, D)
    xq = np.asarray(inputs_q, np.float32)
    xkv = np.asarray(inputs_kv, np.float32)

    in_maps = []
    for cidx in range(N_CORES):
        b, hg = divmod(cidx, H // HPC)
        hs = slice(hg * HW, (hg + 1) * HW)
        xqT = np.ascontiguousarray(xq[b].T)
        xkvT = np.ascontiguousarray(xkv[b].T)
        in_maps.append({
            "xq_bf": xqT[:, :512].astype(BF16),
            "xq_f8": xqT[:, 512:].astype(F8),
            "xkv_bf": xkvT[:, :512].astype(BF16),
            "xkv_f8": xkvT.astype(F8),
            "wq_bf": wq2[:, hs].astype(BF16),
            "wq_f8": wq2[:, hs].astype(F8),
            "wk_bf": wk2[:, hs].astype(BF16),
            "wk_f8": wk2[:, hs].astype(F8),
            "wv_bf": wv2[:, hs].astype(BF16),
            "wv_f8": wv2[:, hs].astype(F8),
            "wo_bf": wo2[hs, :].astype(BF16),
            "wo_f8": wo2[hs, :].astype(F8),
            **consts,
        })

    trace = bool(int(os.environ.get("KERNEL_TRACE", "0")))
    res = bass_utils.run_bass_kernel_spmd(
        nc, in_maps, core_ids=list(range(N_CORES)), trace=trace)
    _CACHE["last_result"] = res

    out = np.zeros((B, S, D), np.float32)
    for cidx in range(N_CORES):
        out[cidx // (H // HPC)] += res.results[cidx]["outp"].astype(np.float32)
    return out
